# revision 3
# baseline (speedup 1.0000x reference)
"""Trainium2 Bass kernel for the cellpose heat-diffusion problem
(nn_Cyto3ONNX, gnn_message_passing).

Reference per iteration: T[meds]+=1; Tneigh = T[n0,n1]*isneighbor;
T[self] = mean(Tneigh); then central-difference gradients.  For the
structured setup_inputs() graph this is a dense masked 9-point stencil
on image rows 1..489:  T_{j+1} = (1/9) L9(T_j) + C,  C = L9(S)/9.

Distribution (x-split, interleaved rows): each of 8 cores owns 256
image columns plus a 32-column recomputed halo on each side (the cut
edge is 489 rows, so halo work is 4x cheaper than a y-split).  The 512
covered rows map to [128 partitions x 4 blocks] via r = 4p + b, so the
dy=+-1 stencil taps become free-dim block offsets; only the block-wrap
rows (b=3 -> p+1 block 0, b=0 -> p-1 block 3) need partition shifts,
done by two tiny shift matmuls per iteration (U/D rows).

x-clip at the true image border is folded into the masks on the host
(the dx=-1 tap at x=0 moves its mask weight onto the dx=0 tap), so no
pad-column maintenance is needed; tile columns outside the image have
zero masks and stay exactly 0.

The seed constant is folded over 3 steps: T3 = C3 = (U^2+U+1)C is the
device start state and every 3rd iteration adds c3 = 9*C3 before the
1/9 scale -- 27 device iterations replace 30, and 2/3 of the C-taps
disappear from the PE stream.

Per-core per-iteration:
  * PE: 2 shift matmuls (U/D wrap rows), then per block b: 9
    accumulating identity matmuls over the products (+ c3 tap every
    3rd iter), PSUM fp32.
  * VectorE/GpSimd: 9 masked products m_k (+) T-views, fp16 2x mode
    (views into tctr / the +1-offset tpad keep reads 4B-aligned).
  * ScalarE: PSUM -> tctr/tpad fp16 copies with scale=1/9, U/D copies.
Gradients: block-shifted subs for dy, tpad-offset sub for dx plus a
maskC*T correction product for the x-clip at the image border.
"""
import numpy as np
from contextlib import ExitStack

LY, LX = 2048, 2048
NPIX = 1_000_000
P0 = LX + 1
OFFS = [(0, 0), (-1, 0), (1, 0), (0, -1), (0, 1),
        (-1, -1), (-1, 1), (1, -1), (1, 1)]
N_CORES = 8
X = 320                  # stored cols per core
HALO = 32
OWN = 256
NB = 4                   # row blocks
ROWS = 512               # rows 0..511 covered; active rows are 1..489
XP = X + 2               # padded block width (image at offset 1)


# ----------------------------------------------------------------- CPU prep
def _dense_masks_folded(isneighbor):
    """[9, ROWS, LX] fp32 masks with the x-clip folded in."""
    d = np.zeros((9, LY * LX), np.float32)
    d[:, P0:P0 + NPIX] = isneighbor.astype(np.float32)
    d = d.reshape(9, LY, LX)[:, :ROWS].copy()
    partner = {3: 0, 5: 1, 7: 2, 4: 0, 6: 1, 8: 2}
    for k in (3, 5, 7):          # dx=-1 taps at x=0 read clip(-1)=0 -> self col
        d[partner[k]][:, 0] += d[k][:, 0]
        d[k][:, 0] = 0.0
    for k in (4, 6, 8):          # dx=+1 taps at x=2047
        d[partner[k]][:, LX - 1] += d[k][:, LX - 1]
        d[k][:, LX - 1] = 0.0
    return d


def _l9p(M, F):
    """Zero-pad masked stencil with folded masks (== clipped L9)."""
    out = np.zeros_like(F)
    for k, (dy, dx) in enumerate(OFFS):
        g = np.zeros_like(F)
        ys0, ys1 = max(dy, 0), min(ROWS + dy, ROWS)
        xs0, xs1 = max(dx, 0), min(LX + dx, LX)
        g[ys0 - dy:ys1 - dy, xs0 - dx:xs1 - dx] = F[ys0:ys1, xs0:xs1]
        out += M[k] * g
    return out


def _seed_image(meds):
    S = np.zeros((ROWS, LX), np.float32)
    np.add.at(S, (meds[:, 0], meds[:, 1]), np.float32(1.0))
    return S


def _to_tiles(A, c):
    """[ROWS, LX] -> [128, NB, X] for core c (zero outside the image)."""
    x0 = OWN * c - HALO
    out = np.zeros((128, NB, X), np.float32)
    a0, a1 = max(x0, 0), min(x0 + X, LX)
    out[:, :, a0 - x0:a1 - x0] = A[:, a0:a1].reshape(128, NB, a1 - a0)
    return out


def _shift_mats():
    ident = np.eye(128)
    wu = np.eye(128, k=-1)      # out[p] = rhs[p+1]
    wd = np.eye(128, k=1)       # out[p] = rhs[p-1]
    return np.stack([ident, wu, wd]).astype(np.float16)


def _schedule(niter):
    """Device-iteration tap schedule. Returns (start_is_c3, taps) where
    taps[i] in {None, 'c3', 'c9'}."""
    if niter >= 3:
        m = niter - 3
        taps = []
        for i in range(m - m % 3):
            taps.append('c3' if i % 3 == 2 else None)
        taps += ['c9'] * (m % 3)
        return True, taps
    return False, ['c9'] * niter


def _prep_in_maps(isneighbor, meds):
    M = _dense_masks_folded(isneighbor)
    S = _seed_image(np.asarray(meds))
    c9 = _l9p(M, S)
    u1 = _l9p(M, c9) / 9.0
    u2 = _l9p(M, u1) / 9.0
    c3 = u2 + u1 + c9                     # 9 * C3
    mats = _shift_mats()
    in_maps = []
    for c in range(N_CORES):
        mk = np.stack([_to_tiles(M[k], c) for k in range(9)]).astype(np.float16)
        c3t = _to_tiles(c3, c).astype(np.float16)
        c9t = _to_tiles(c9, c).astype(np.float16)
        mc = np.zeros((128, NB, X), np.float16)
        x0 = OWN * c - HALO
        if 0 >= x0 and 0 < x0 + X:
            mc[:, :, 0 - x0] = -1.0
        if LX - 1 < x0 + X:
            mc[:, :, LX - 1 - x0] = 1.0
        in_maps.append({"masks": mk, "c3": c3t, "c9": c9t,
                        "maskc": mc, "mats": mats})
    return in_maps, c3


def _start_tiles(in_maps, start_is_c3):
    for pc in in_maps:
        t0 = (pc["c3"].astype(np.float32) / 9.0).astype(np.float16) \
            if start_is_c3 else np.zeros((128, NB, X), np.float16)
        pc["t0ctr"] = t0
        t0p = np.zeros((128, NB, XP), np.float16)
        t0p[:, :, 1:X + 1] = t0
        pc["t0pad"] = t0p
    return in_maps


# ------------------------------------------------------------- bass program
def _build_bass(niter, mode="full"):
    import concourse.bass as bass
    import concourse.bacc as bacc
    import concourse.tile as tile
    import concourse.mybir as mybir

    f16, f32 = mybir.dt.float16, mybir.dt.float32
    nc = bacc.Bacc("TRN2", target_bir_lowering=False, debug=False,
                   num_devices=N_CORES)
    d_masks = nc.dram_tensor("masks", [9, 128, NB, X], f16, kind="ExternalInput").ap()
    d_c3 = nc.dram_tensor("c3", [128, NB, X], f16, kind="ExternalInput").ap()
    d_c9 = nc.dram_tensor("c9", [128, NB, X], f16, kind="ExternalInput").ap()
    d_maskc = nc.dram_tensor("maskc", [128, NB, X], f16, kind="ExternalInput").ap()
    d_mats = nc.dram_tensor("mats", [3, 128, 128], f16, kind="ExternalInput").ap()
    d_t0ctr = nc.dram_tensor("t0ctr", [128, NB, X], f16, kind="ExternalInput").ap()
    d_t0pad = nc.dram_tensor("t0pad", [128, NB, XP], f16, kind="ExternalInput").ap()
    d_mu = nc.dram_tensor("mu", [2, 128, NB, X], f32, kind="ExternalOutput").ap()

    start_is_c3, taps = _schedule(niter)
    ndev = len(taps)
    inv9 = float(np.float32(1.0) / np.float32(9.0))
    # tap k -> (dy, dx); engine split: Pool takes k=3 main and the two
    # misaligned dx=0 wrap products (U/D image offset 1 breaks 2x on DVE)
    POOL_MAIN = (3,)
    POOL_WRAP = (1, 2)

    with ExitStack() as ctx:
        tc = ctx.enter_context(tile.TileContext(nc))
        const = ctx.enter_context(tc.tile_pool(name="const", bufs=1))
        state = ctx.enter_context(tc.tile_pool(name="state", bufs=1))
        prods = ctx.enter_context(tc.tile_pool(name="prods", bufs=2))
        psum = ctx.enter_context(tc.tile_pool(name="psum", bufs=1, space="PSUM"))

        mask_t = []
        for k in range(9):
            mt = const.tile([128, NB, X], f16, tag=f"mask{k}", name=f"mask{k}")
            if mode == "noload":
                nc.vector.memset(mt[:], 0.5)
            else:
                for b in range(NB):
                    nc.sync.dma_start(mt[:, b, :], d_masks[k, :, b])
            mask_t.append(mt)
        c3_t = const.tile([128, NB, X], f16, tag="c3", name="c3t")
        c9_t = const.tile([128, NB, X], f16, tag="c9", name="c9t")
        mc_t = const.tile([128, NB, X], f16, tag="mc", name="mct")
        mats_t = const.tile([128, 3 * 128], f16, tag="mats", name="matst")
        tctr = [state.tile([128, NB, X], f16, tag=f"tctr{i}", name=f"tctr{i}")
                for i in range(2)]
        tpad = [state.tile([128, NB, XP], f16, tag=f"tpad{i}", name=f"tpad{i}")
                for i in range(2)]
        if mode == "noload":
            for t in (c3_t, c9_t, mc_t, tctr[0], tpad[0]):
                nc.vector.memset(t[:], 0.01)
            nc.vector.memset(mats_t[:], 0.0)
        else:
            nc.sync.dma_start(c3_t[:], d_c3[:])
            nc.sync.dma_start(c9_t[:], d_c9[:])
            nc.sync.dma_start(mc_t[:], d_maskc[:])
            for j in range(3):
                nc.sync.dma_start(mats_t[:, j * 128:(j + 1) * 128], d_mats[j])
            nc.sync.dma_start(tctr[0][:], d_t0ctr[:])
            nc.sync.dma_start(tpad[0][:], d_t0pad[:])

        def lhsT(j):
            return mats_t[:, j * 128: j * 128 + 128]

        def make_ud(cur):
            """U/D wrap rows of tpad[cur] -> fp16 [128, XP] tiles."""
            up = psum.tile([128, XP], f32, tag="upp", name="upp")
            dp = psum.tile([128, XP], f32, tag="dpp", name="dpp")
            nc.tensor.matmul(up[:], lhsT(1), tpad[cur][:, 0, :],
                             start=True, stop=True)
            nc.tensor.matmul(dp[:], lhsT(2), tpad[cur][:, NB - 1, :],
                             start=True, stop=True)
            ut = prods.tile([128, XP], f16, tag="upad", name="upad")
            dt = prods.tile([128, XP], f16, tag="dpad", name="dpad")
            nc.scalar.copy(ut[:], up[:])
            nc.scalar.copy(dt[:], dp[:])
            return ut, dt

        for i in range(ndev):
            cur, nxt = i % 2, (i + 1) % 2
            if mode != "full":
                cur, nxt = 0, 1
            ut, dt = make_ud(cur)

            def tview(dy, dx, b0, b1):
                """T source view for out-blocks [b0:b1), read blocks +dy."""
                if dx == 0:
                    return tctr[cur][:, b0 + dy:b1 + dy, :]
                return tpad[cur][:, b0 + dy:b1 + dy, 1 + dx:1 + X + dx]

            pk = []
            for k in range(9):
                pk.append(prods.tile([128, NB, X], f16, tag=f"prod{k}",
                                     name=f"prod{k}"))
            # main parts
            for k, (dy, dx) in enumerate(OFFS):
                eng = nc.gpsimd if k in POOL_MAIN else nc.vector
                if dy == 0:
                    eng.tensor_mul(pk[k][:], mask_t[k][:], tview(0, dx, 0, NB))
                elif dy == 1:
                    eng.tensor_mul(pk[k][:, 0:NB - 1, :],
                                   mask_t[k][:, 0:NB - 1, :],
                                   tview(1, dx, 0, NB - 1))
                else:
                    eng.tensor_mul(pk[k][:, 1:NB, :],
                                   mask_t[k][:, 1:NB, :],
                                   tview(-1, dx, 1, NB))
            # wrap parts (need U/D)
            for k, (dy, dx) in enumerate(OFFS):
                if dy == 0:
                    continue
                eng = nc.gpsimd if k in POOL_WRAP else nc.vector
                if dy == 1:
                    eng.tensor_mul(pk[k][:, NB - 1, :], mask_t[k][:, NB - 1, :],
                                   ut[:, 1 + dx:1 + X + dx])
                else:
                    eng.tensor_mul(pk[k][:, 0, :], mask_t[k][:, 0, :],
                                   dt[:, 1 + dx:1 + X + dx])
            if mode == "noPE":
                continue
            # accumulate + copy back, per block
            order = [0, 1, 2, 3]
            for b in order:
                acc = psum.tile([128, X], f32, tag=f"acc{b}", name=f"acc{b}")
                seq = [pk[k][:, b, :] for k in range(9)]
                if taps[i] == 'c3':
                    seq.append(c3_t[:, b, :])
                elif taps[i] == 'c9':
                    seq.append(c9_t[:, b, :])
                for j, rhs in enumerate(seq):
                    nc.tensor.matmul(acc[:], lhsT(0), rhs,
                                     start=(j == 0), stop=(j == len(seq) - 1))
                if mode == "noACT":
                    continue
                nc.scalar.mul(tpad[nxt][:, b, 1:1 + X], acc[:], inv9)
                nc.scalar.mul(tctr[nxt][:, b, :], acc[:], inv9)

        fin = ndev % 2
        # ------- gradients -------
        ut, dt = make_ud(fin)
        dy_s = state.tile([128, NB, X], f32, tag="dys", name="dys")
        dx_s = state.tile([128, NB, X], f32, tag="dxs", name="dxs")
        g2 = state.tile([128, NB, X], f32, tag="g2", name="g2")
        T, P = tctr[fin], tpad[fin]
        nc.vector.tensor_sub(dy_s[:, 1:3, :], T[:, 2:4, :], T[:, 0:2, :])
        nc.vector.tensor_sub(dy_s[:, 0, :], T[:, 1, :], dt[:, 1:1 + X])
        nc.vector.tensor_sub(dy_s[:, 3, :], ut[:, 1:1 + X], T[:, 2, :])
        nc.gpsimd.tensor_mul(g2[:], mc_t[:], T[:])
        nc.vector.tensor_sub(dx_s[:], P[:, :, 2:2 + X], P[:, :, 0:X])
        nc.vector.tensor_add(dx_s[:], dx_s[:], g2[:])
        nc.sync.dma_start(d_mu[0], dy_s[:])
        nc.sync.dma_start(d_mu[1], dx_s[:])
    return nc


# ------------------------------------------------------------------ runner
_CACHE = {}


def _pjrt_exec(nc):
    """Finalize nc and build a reusable jitted 8-core SPMD executable."""
    import jax
    import concourse.mybir as mybir
    from concourse import bass2jax
    from jax.sharding import Mesh, PartitionSpec
    from jax.experimental.shard_map import shard_map

    nc.finalize()
    bass2jax.install_neuronx_cc_hook()

    part_name = nc.partition_id_tensor.name if nc.partition_id_tensor else None
    in_names, out_names, out_avals, zero_outs = [], [], [], []
    for alloc in nc.m.functions[0].allocations:
        if not isinstance(alloc, mybir.MemoryLocationSet):
            continue
        name = alloc.memorylocations[0].name
        if alloc.kind == "ExternalInput":
            if name != part_name:
                in_names.append(name)
        elif alloc.kind == "ExternalOutput":
            out_names.append(name)
            shape = tuple(alloc.tensor_shape)
            dtype = mybir.dt.np(alloc.dtype)
            out_avals.append(jax.core.ShapedArray(shape, dtype))
            zero_outs.append(np.zeros(shape, dtype))
    n_params = len(in_names)
    all_names = in_names + out_names
    if part_name is not None:
        all_names = all_names + [part_name]

    def _body(*args):
        operands = list(args)
        if part_name is not None:
            operands.append(bass2jax.partition_id_tensor())
        outs = bass2jax._bass_exec_p.bind(
            *operands,
            out_avals=tuple(out_avals),
            in_names=tuple(all_names),
            out_names=tuple(out_names),
            lowering_input_output_aliases=(),
            sim_require_finite=True,
            sim_require_nnan=True,
            nc=nc,
        )
        return tuple(outs)

    devices = jax.devices()[:N_CORES]
    mesh = Mesh(np.asarray(devices), ("core",))
    specs = (PartitionSpec("core"),) * (n_params + len(out_names))
    sharded = jax.jit(
        shard_map(_body, mesh=mesh, in_specs=specs,
                  out_specs=(PartitionSpec("core"),) * len(out_names),
                  check_rep=False),
        keep_unused=True,
    )

    def stage(in_maps):
        concat = [np.concatenate([np.asarray(in_maps[c][n]) for c in range(N_CORES)],
                                 axis=0) for n in in_names]
        concat += [np.concatenate([z] * N_CORES, axis=0) for z in zero_outs]
        return concat

    def run(in_maps, device_inputs=None):
        if device_inputs is None:
            device_inputs = stage(in_maps)
        out_arrs = sharded(*device_inputs)
        return [
            {name: np.asarray(out_arrs[i]).reshape(N_CORES, *out_avals[i].shape)[c]
             for i, name in enumerate(out_names)}
            for c in range(N_CORES)
        ]

    return run, stage, sharded, in_names, out_names, mesh


def _get_runner(niter):
    key = int(niter)
    if key not in _CACHE:
        _CACHE[key] = _pjrt_exec(_build_bass(key))
    return _CACHE[key]


# ---------------------------------------------------------------- fallback
def _fallback(neighbors, isneighbor, meds, T, niter):
    m0, m1 = meds[:, 0], meds[:, 1]
    n0, n1 = neighbors[0], neighbors[1]
    T = np.array(T, np.float32, copy=True)
    isn = isneighbor.astype(np.float32)
    for _ in range(int(niter)):
        np.add.at(T, (m0, m1), np.float32(1.0))
        Tneigh = T[n0, n1] * isn
        T[n0[0], n1[0]] = np.mean(Tneigh, axis=0, dtype=np.float32)
    idx = np.array([2, 1, 4, 3])
    grads = T[n0[idx], n1[idx]]
    return np.stack((grads[0] - grads[1], grads[2] - grads[3]),
                    axis=-2).astype(np.float32)


def _fast_path_ok(neighbors, isneighbor, meds, T, niter):
    if neighbors.shape != (2, 9, NPIX) or isneighbor.shape != (9, NPIX):
        return False
    if T.shape != (LY, LX) or meds.ndim != 2 or meds.shape[1] != 2:
        return False
    if T.any():
        return False
    mf = meds[:, 0].astype(np.int64) * LX + meds[:, 1]
    if mf.min() < P0 or mf.max() >= P0 + NPIX:
        return False
    flat = np.arange(NPIX, dtype=np.int64) + P0
    y = (flat // LX).astype(np.int32)
    x = (flat % LX).astype(np.int32)
    offs = np.array(OFFS, np.int32)
    n0e = np.clip(y[None, :] + offs[:, 0:1], 0, LY - 1)
    n1e = np.clip(x[None, :] + offs[:, 1:2], 0, LX - 1)
    return (np.array_equal(neighbors[0], n0e)
            and np.array_equal(neighbors[1], n1e))


# ------------------------------------------------------------------- entry
def kernel(neighbors, isneighbor, meds, T, niter):
    neighbors = np.asarray(neighbors)
    isneighbor = np.asarray(isneighbor)
    meds = np.asarray(meds)
    T = np.asarray(T)
    ni = int(np.asarray(niter))
    if not _fast_path_ok(neighbors, isneighbor, meds, T, ni):
        return _fallback(neighbors, isneighbor, meds, T, ni)

    try:
        in_maps, _ = _prep_in_maps(isneighbor, meds)
        start_is_c3, _ = _schedule(ni)
        in_maps = _start_tiles(in_maps, start_is_c3)
        run, stage, _, _, _, _ = _get_runner(ni)
        results = run(in_maps)
        mu = np.zeros((2, ROWS, LX), np.float32)
        for c in range(N_CORES):
            m = results[c]["mu"][:, :, :, HALO:HALO + OWN]   # [2,128,4,256]
            mu[:, :, OWN * c:OWN * (c + 1)] = m.reshape(2, ROWS, OWN)
        out = mu.reshape(2, ROWS * LX)[:, P0:P0 + NPIX]
        out = np.ascontiguousarray(out.astype(np.float32))
        if not np.isfinite(out).all() or np.abs(out).max() > 1e6:
            raise RuntimeError("implausible kernel output")
        return out
    except Exception:
        import os
        if os.environ.get("BASSK_RAISE", "0") == "1":
            raise
        return _fallback(neighbors, isneighbor, meds, T, ni)
